# revision 6
# baseline (speedup 1.0000x reference)
"""Multi-head attention (batch=2, seq=2048, d_model=1024, 16 heads, causal, RoPE)
on 8 Trainium2 NeuronCores.

Sharding: core i handles batch b = i//4 and head group g = i%4 (4 heads each).
Per core, everything is computed in "transposed score space":
  - host supplies x^T (so QKV projections need no on-chip transpose)
  - Q^T, K^T come out of the PE with head_dim on partitions; RoPE is applied
    with a host-side de-interleaving column permutation of qw/kw (dot products
    are invariant since the same permutation is applied to both Q and K)
  - S^T = K^T.T @ Q^T (contraction over head_dim), exp on ScalarE with
    scale=1/32 and no max-subtraction (scores are provably < 1 in magnitude;
    softmax is shift-invariant anyway), causal masking via 0/1 mask multiply
  - PV uses V in natural layout as lhsT, augmented with a ones column per head
    so the softmax denominator l falls out of the same matmuls (psum row 64)
  - normalization: linv = 1/l on VectorE, broadcast across partitions with a
    K=1 outer-product matmul, then one multiply
  - output projection consumes O^T directly as lhsT (no transpose), partial
    result (rows of ow summed only over this core's heads) is DMA'd out
The host sums the 4 partial outputs per batch (the "all-reduce" of the
sharding hint, done at gather time).
"""
import numpy as np

import concourse.bass as bass
import concourse.tile as tile
from concourse import bacc, mybir
from concourse.bass import ts
from concourse.bass_utils import run_bass_kernel_spmd

F16 = mybir.dt.float16
F32 = mybir.dt.float32
AF = mybir.ActivationFunctionType

B, SEQ, D = 2, 2048, 1024
NH, HD = 16, 64
HG = 4            # heads per core
DH = HG * HD      # 256
N_CORES = 8
P = 128

PERM = np.concatenate([np.arange(0, 64, 2), np.arange(1, 64, 2)])


def build_program():
    nc = bacc.Bacc("TRN2", target_bir_lowering=False, debug=False,
                   num_devices=N_CORES)

    xT = nc.dram_tensor("xT", [P, 8, SEQ], F16, kind="ExternalInput").ap()
    wq = nc.dram_tensor("wq", [P, 8, DH], F16, kind="ExternalInput").ap()
    wk = nc.dram_tensor("wk", [P, 8, DH], F16, kind="ExternalInput").ap()
    wv = nc.dram_tensor("wv", [P, 8, DH], F16, kind="ExternalInput").ap()
    wo = nc.dram_tensor("wo", [P, 2, D], F16, kind="ExternalInput").ap()
    cosT = nc.dram_tensor("cosT", [P, SEQ], F16, kind="ExternalInput").ap()
    sinT = nc.dram_tensor("sinT", [P, SEQ], F16, kind="ExternalInput").ap()
    masks = nc.dram_tensor("masks", [P, 4, 512], F16, kind="ExternalInput").ap()
    ones = nc.dram_tensor("ones", [1, 65], F32, kind="ExternalInput").ap()
    out = nc.dram_tensor("out", [SEQ, D], F32, kind="ExternalOutput").ap()

    with tile.TileContext(nc) as tc:
        _build(tc, xT, wq, wk, wv, wo, cosT, sinT, masks, ones, out)

    nc.compile()
    return nc


def _build(tc, xT, wq, wk, wv, wo, cosT, sinT, masks, ones, out):
    nc = tc.nc
    from contextlib import ExitStack
    ctx = ExitStack()

    consts = ctx.enter_context(tc.tile_pool(name="consts", bufs=1))
    raw = ctx.enter_context(tc.tile_pool(name="raw", bufs=2))
    tmp = ctx.enter_context(tc.tile_pool(name="tmpp", bufs=2))
    ptp = ctx.enter_context(tc.tile_pool(name="ptp", bufs=3))
    lrow = ctx.enter_context(tc.tile_pool(name="lrow", bufs=2))
    outs = ctx.enter_context(tc.tile_pool(name="outs", bufs=3))
    ps_sg = ctx.enter_context(tc.tile_pool(name="ps_sg", bufs=2, space="PSUM"))
    ps_acc = ctx.enter_context(tc.tile_pool(name="ps_acc", bufs=2, space="PSUM"))
    ps_bc = ctx.enter_context(tc.tile_pool(name="ps_bc", bufs=1, space="PSUM"))

    # persistent SBUF tensors
    xT_sb = consts.tile([P, 8, SEQ], F16, tag="xT_sb")
    nc.sync.dma_start(xT_sb[:], xT[:])
    wq_sb = consts.tile([P, 8, DH], F16, tag="wq_sb")
    nc.sync.dma_start(wq_sb[:], wq[:])
    wk_sb = consts.tile([P, 8, DH], F16, tag="wk_sb")
    nc.sync.dma_start(wk_sb[:], wk[:])
    wv_sb = consts.tile([P, 8, DH], F16, tag="wv_sb")
    nc.sync.dma_start(wv_sb[:], wv[:])
    wo_sb = consts.tile([P, 2, D], F16, tag="wo_sb")
    nc.sync.dma_start(wo_sb[:], wo[:])
    cos_sb = consts.tile([P, SEQ], F16, tag="cos_sb")
    nc.sync.dma_start(cos_sb[:], cosT[:])
    sin_sb = consts.tile([P, SEQ], F16, tag="sin_sb")
    nc.sync.dma_start(sin_sb[:], sinT[:])
    mask_sb = consts.tile([P, 4, 512], F16, tag="mask_sb")
    nc.sync.dma_start(mask_sb[:], masks[:])
    ones_sb = consts.tile([1, 65], F32, tag="ones_sb")
    nc.sync.dma_start(ones_sb[:], ones[:])

    qt0 = consts.tile([P, SEQ], F16, tag="qt0")
    qt1 = consts.tile([P, SEQ], F16, tag="qt1")
    kt0 = consts.tile([P, SEQ], F16, tag="kt0")
    kt1 = consts.tile([P, SEQ], F16, tag="kt1")
    QT = [qt0, qt1]
    KT = [kt0, kt1]
    on0 = consts.tile([P, SEQ], F16, tag="on0")
    on1 = consts.tile([P, SEQ], F16, tag="on1")
    ON = [on0, on1]
    vaug = consts.tile([P, 16, 260], F16, tag="vaug")

    # preload the exp table set while the QKV matmuls run
    dmy = consts.tile([1, 1], F32, tag="dmy")
    nc.scalar.activation(dmy[:], ones_sb[0:1, 0:1], AF.Exp)

    # ---- Phase 1: QKV projections -------------------------------------
    # Q^T / K^T: [dh, q] head-pair chunks, then RoPE into QT/KT tiles.
    for wsb, dst in ((wq_sb, QT), (wk_sb, KT)):
        for m in range(2):
            rw = raw.tile([P, SEQ], F16, tag="rw")
            for qp in range(2):          # pairs of q-chunks share a psum tile
                pm = ps_sg.tile([P, 1024], F32, tag="sg")
                for half in range(2):
                    qc = 2 * qp + half
                    for c in range(8):
                        nc.tensor.matmul(
                            pm[:, ts(half, 512)],
                            wsb[:, c, ts(m, 128)],
                            xT_sb[:, c, ts(qc, 512)],
                            start=(c == 0), stop=(c == 7),
                        )
                    nc.scalar.copy(rw[:, ts(qc, 512)], pm[:, ts(half, 512)])
            # RoPE: rows [a.x1 | a.x2 | b.x1 | b.x2] (32 each).
            # rws = rw with x1/x2 swapped within each 64-row head block
            # (partition moves via DMA — DVE can't mix base partitions).
            rws = tmp.tile([P, SEQ], F16, tag="rws")
            for b0 in (0, 64):
                nc.sync.dma_start(rws[b0:b0 + 32, :], rw[b0 + 32:b0 + 64, :])
                nc.sync.dma_start(rws[b0 + 32:b0 + 64, :], rw[b0:b0 + 32, :])
            t1 = tmp.tile([P, SEQ], F16, tag="t1")
            t2 = tmp.tile([P, SEQ], F16, tag="t2")
            nc.vector.tensor_mul(t1[:], rw[:], cos_sb[:])
            nc.vector.tensor_mul(t2[:], rws[:], sin_sb[:])
            nc.vector.tensor_add(dst[m][:], t1[:], t2[:])

    # V: natural [k, dh], written into vaug with a ones column per head
    nc.gpsimd.memset(vaug[:], 1.0)
    for kp in range(8):                 # pairs of k-chunks
        pv = ps_sg.tile([P, 1024], F32, tag="sg")
        for half in range(2):
            kc = 2 * kp + half
            for c in range(8):
                nc.tensor.matmul(
                    pv[:, ts(half, 512)][:, 0:DH],
                    xT_sb[:, c, ts(kc, 128)],
                    wv_sb[:, c, :],
                    start=(c == 0), stop=(c == 7),
                )
            nc.vector.tensor_copy(
                vaug[:, kc].rearrange("p (h c) -> p h c", c=65)[:, :, 0:64],
                pv[:, ts(half, 512)][:, 0:DH].rearrange("p (h c) -> p h c", c=64),
            )

    # ---- Phase 2: attention (scores^T -> exp -> mask -> PV) -----------
    for h in range(HG):
        r0 = 64 * (h % 2)
        kth = KT[h // 2][r0:r0 + 64, :]
        qth = QT[h // 2][r0:r0 + 64, :]
        for qc in range(4):
            n_k = 4 * (qc + 1)
            acc = ps_acc.tile([65, 512], F32, tag="acc")
            groups = []
            for gi in range(n_k // 2):
                sg = ps_sg.tile([P, 1024], F32, tag="sg")
                for d in range(2):
                    j = 2 * gi + d
                    nc.tensor.matmul(
                        sg[:, ts(d, 512)],
                        kth[:, ts(j, 128)],
                        qth[:, ts(qc, 512)],
                        start=True, stop=True,
                    )
                groups.append((gi, sg))
                # software-pipeline: process previous group while this one runs
                if len(groups) == 2:
                    _attn_group(nc, ptp, mask_sb, vaug, acc, groups.pop(0),
                                h, qc, n_k)
            _attn_group(nc, ptp, mask_sb, vaug, acc, groups.pop(0), h, qc, n_k)

            # normalization (only one TT operand may live in PSUM)
            lt = lrow.tile([1, 512], F32, tag="lr")
            nc.vector.reciprocal(lt[:], acc[64:65, :])
            bc = ps_bc.tile([65, 512], F32, tag="bc")
            nc.tensor.matmul(bc[:], ones_sb[:], lt[:], start=True, stop=True)
            oacc = lrow.tile([64, 512], F16, tag="oacc")
            nc.vector.tensor_copy(oacc[:], acc[0:64, :])
            nc.vector.tensor_mul(ON[h // 2][r0:r0 + 64, ts(qc, 512)],
                                 oacc[:], bc[0:64, :])

    # ---- Phase 3: output projection -----------------------------------
    for q16 in range(16):
        po = ps_sg.tile([P, 1024], F32, tag="sg")
        for n in range(2):
            for c in range(2):
                nc.tensor.matmul(
                    po[:, ts(n, 512)],
                    ON[c][:, ts(q16, 128)],
                    wo_sb[:, c, ts(n, 512)],
                    start=(c == 0), stop=(c == 1),
                )
        ob = outs.tile([P, D], F32, tag="ob")
        nc.vector.tensor_copy(ob[:, 0:512], po[:, 0:512])
        nc.scalar.copy(ob[:, 512:1024], po[:, 512:1024])
        nc.sync.dma_start(out[ts(q16, 128), :], ob[:])

    ctx.close()


def _attn_group(nc, ptp, mask_sb, vaug, acc, group, h, qc, n_k):
    gi, sg = group
    pt = ptp.tile([P, 1024], F16, tag="pt")
    nc.scalar.activation(pt[:], sg[:], AF.Exp, scale=1.0 / 32.0)
    for d in range(2):
        j = 2 * gi + d
        v = j - 4 * qc
        if v >= 0:
            nc.vector.tensor_mul(pt[:, ts(d, 512)], pt[:, ts(d, 512)],
                                 mask_sb[:, v, :])
    for d in range(2):
        j = 2 * gi + d
        nc.tensor.matmul(
            acc[:],
            vaug[:, j, 65 * h:65 * h + 65],
            pt[:, ts(d, 512)],
            start=(j == 0), stop=(j == n_k - 1),
        )


# ---------------------------------------------------------------------------
# host side
# ---------------------------------------------------------------------------

def _host_tables():
    inv = 1.0 / (10000.0 ** (np.arange(0, HD, 2, dtype=np.float64) / HD))
    pos = np.arange(SEQ, dtype=np.float64)
    freq = pos[None, :] * inv[:, None]            # (32, 2048)
    cos32 = np.cos(freq)
    sin32 = np.sin(freq)
    cosT = np.tile(cos32, (4, 1)).astype(np.float16)
    # sign folded for the swapped operand: rows multiplying x2 (the x1 output
    # rows) get -sin, rows multiplying x1 (the x2 output rows) get +sin
    sinT = np.concatenate([-sin32, sin32, -sin32, sin32], 0).astype(np.float16)
    return np.ascontiguousarray(cosT), np.ascontiguousarray(sinT)


def _host_masks():
    m = np.zeros((P, 4, 512), dtype=np.float16)
    kk = np.arange(P)[:, None]
    qq = np.arange(512)[None, :]
    for v in range(4):
        m[:, v, :] = (qq >= kk + 128 * v).astype(np.float16)
    return np.ascontiguousarray(m)


_NC = None


def _get_program():
    global _NC
    if _NC is None:
        _NC = build_program()
    return _NC


def kernel(x, qw, kw, vw, ow):
    out, _ = _run(x, qw, kw, vw, ow)
    return out


def _run(x, qw, kw, vw, ow, trace=False):
    x = np.asarray(x, dtype=np.float16)
    qw = np.asarray(qw, dtype=np.float16)
    kw = np.asarray(kw, dtype=np.float16)
    vw = np.asarray(vw, dtype=np.float16)
    ow = np.asarray(ow, dtype=np.float16)

    cosT, sinT = _host_tables()
    masks = _host_masks()
    ones = np.ones((1, 65), dtype=np.float32)

    def shard_w(w):  # (1024, 256) -> [128, 8, 256]
        return np.ascontiguousarray(w.reshape(8, P, DH).transpose(1, 0, 2))

    in_maps = []
    for core in range(N_CORES):
        b, g = core // 4, core % 4
        xT = x[b].T                                   # (1024, 2048)
        xT_t = np.ascontiguousarray(
            xT.reshape(8, P, SEQ).transpose(1, 0, 2))  # [128, 8, 2048]
        cols = np.concatenate([64 * (4 * g + h) + PERM for h in range(HG)])
        wo_g = ow[DH * g:DH * (g + 1), :]              # (256, 1024)
        wo_t = np.ascontiguousarray(
            wo_g.reshape(2, P, D).transpose(1, 0, 2))  # [128, 2, 1024]
        in_maps.append({
            "xT": xT_t,
            "wq": shard_w(qw[:, cols]),
            "wk": shard_w(kw[:, cols]),
            "wv": shard_w(vw[:, DH * g:DH * (g + 1)]),
            "wo": wo_t,
            "cosT": cosT,
            "sinT": sinT,
            "masks": masks,
            "ones": ones,
        })

    nc = _get_program()
    res = run_bass_kernel_spmd(nc, in_maps, core_ids=list(range(N_CORES)),
                               trace=trace)

    out = np.zeros((B, SEQ, D), dtype=np.float32)
    for core in range(N_CORES):
        out[core // 4] += res.results[core]["out"]
    return out, res
